# revision 3
# baseline (speedup 1.0000x reference)
"""Trainium2 Bass kernel for multi-head attention with cross-head renormalization.

Reference computation (b=4, n=2048, d=512, h=8, hd=64):
    Q = x @ Wq.T ; K = x @ Wk.T ; V = x @ Wv.T          (per head split)
    S = (Q @ K.T) * sqrt(hd)                            # SCALE = 8
    P = softmax(S, axis=-1)
    P = P / P.sum(axis=heads) + 1e-6                    # cross-head renorm
    out = (P @ V) @ Wo.T

Sharding: 8 cores = 4 batch elements x 2 query-halves. Each core computes
K/V for the full 2048-token sequence of its batch element (replicated
within the pair) and Q/outputs for its own 1024 queries. The cross-head
renorm is then fully core-local; no collectives.

Device layout notes (per core):
  - All matmul operands bf16; fp32 PSUM accumulation.
  - x^T, weights^T are prepared host-side (transpose + bf16 cast).
  - SCALE is folded into Wq host-side.
  - The +EPS term is applied as out += EPS * colsum(V) via the bias input
    of the PSUM->SBUF copy of the attention output.
"""

import sys
import numpy as np

for _p in ("/opt/trn_rl_repo", "/root/.axon_site/_ro/trn_rl_repo"):
    if _p not in sys.path:
        sys.path.append(_p)

import ml_dtypes
from contextlib import ExitStack

import concourse.bass as bass
import concourse.tile as tile
from concourse import bacc, mybir
from concourse.bass_utils import run_bass_kernel_spmd

BF16 = mybir.dt.bfloat16
F32 = mybir.dt.float32
AF = mybir.ActivationFunctionType
OP = mybir.AluOpType

N = 2048          # sequence length (keys)
D = 512           # model dim
H = 8             # heads
HD = 64           # head dim
NQ = 1024         # queries per core
QC = NQ // 128    # query chunks per core (8)
KT = N // 128     # key tiles (16)
CT = D // 128     # contraction tiles (4)
SCALE = 8.0       # sqrt(HD)
EPS = 1e-6

_CACHED_NC = None


def build():
    nc = bacc.Bacc("TRN2", target_bir_lowering=False, debug=False)

    xT = nc.dram_tensor("xT", [CT, 128, N], BF16, kind="ExternalInput")
    xqT = nc.dram_tensor("xqT", [CT, 128, NQ], BF16, kind="ExternalInput")
    wq = nc.dram_tensor("wq", [CT, 128, D], BF16, kind="ExternalInput")
    wk = nc.dram_tensor("wk", [CT, 128, D], BF16, kind="ExternalInput")
    wv = nc.dram_tensor("wv", [CT, 128, D], BF16, kind="ExternalInput")
    wo = nc.dram_tensor("wo", [CT, 128, D], BF16, kind="ExternalInput")
    y = nc.dram_tensor("y", [QC, 128, D], F32, kind="ExternalOutput")

    with tile.TileContext(nc) as tc, ExitStack() as ctx:
        sb = ctx.enter_context(tc.tile_pool(name="sb", bufs=1))
        pbuf = ctx.enter_context(tc.tile_pool(name="pbuf", bufs=2))
        ptbuf = ctx.enter_context(tc.tile_pool(name="ptbuf", bufs=2))
        big_ps = ctx.enter_context(tc.tile_pool(name="big_ps", bufs=1, space="PSUM"))
        med_ps = ctx.enter_context(tc.tile_pool(name="med_ps", bufs=2, space="PSUM"))
        ot_ps_pool = ctx.enter_context(tc.tile_pool(name="ot_ps", bufs=1, space="PSUM"))

        # ---- load inputs ----
        xT_sb = sb.tile([128, CT, N], BF16)
        xqT_sb = sb.tile([128, CT, NQ], BF16)
        wq_sb = sb.tile([128, CT, D], BF16)
        wk_sb = sb.tile([128, CT, D], BF16)
        wv_sb = sb.tile([128, CT, D], BF16)
        wo_sb = sb.tile([128, CT, D], BF16)
        for ci in range(CT):
            nc.sync.dma_start(xT_sb[:, ci, :], xT.ap()[ci])
            nc.sync.dma_start(xqT_sb[:, ci, :], xqT.ap()[ci])
            nc.sync.dma_start(wq_sb[:, ci, :], wq.ap()[ci])
            nc.sync.dma_start(wk_sb[:, ci, :], wk.ap()[ci])
            nc.sync.dma_start(wv_sb[:, ci, :], wv.ap()[ci])
            nc.sync.dma_start(wo_sb[:, ci, :], wo.ap()[ci])

        # ---- projections ----
        QT_sb = sb.tile([128, CT, NQ], BF16)   # Q^T (o on partitions), scaled by 8
        KT_sb = sb.tile([128, CT, N], BF16)    # K^T
        V_sb = sb.tile([128, KT, D], BF16)     # V (k on partitions)

        for oj in range(CT):
            q_ps = big_ps.tile([128, N], F32, tag="bigps")
            for ci in range(CT):
                for nh in range(NQ // 512):
                    nc.tensor.matmul(
                        q_ps[:, nh * 512:(nh + 1) * 512],
                        wq_sb[:, ci, oj * 128:(oj + 1) * 128],
                        xqT_sb[:, ci, nh * 512:(nh + 1) * 512],
                        start=(ci == 0), stop=(ci == CT - 1),
                    )
            nc.scalar.copy(QT_sb[:, oj, :], q_ps[:, :NQ])

            k_ps = big_ps.tile([128, N], F32, tag="bigps")
            for ci in range(CT):
                for nh in range(N // 512):
                    nc.tensor.matmul(
                        k_ps[:, nh * 512:(nh + 1) * 512],
                        wk_sb[:, ci, oj * 128:(oj + 1) * 128],
                        xT_sb[:, ci, nh * 512:(nh + 1) * 512],
                        start=(ci == 0), stop=(ci == CT - 1),
                    )
            nc.scalar.copy(KT_sb[:, oj, :], k_ps[:])

        for kt in range(KT):
            v_ps = med_ps.tile([128, D], F32, tag="medps")
            for ci in range(CT):
                nc.tensor.matmul(
                    v_ps[:],
                    xT_sb[:, ci, kt * 128:(kt + 1) * 128],
                    wv_sb[:, ci, :],
                    start=(ci == 0), stop=(ci == CT - 1),
                )
            nc.scalar.copy(V_sb[:, kt, :], v_ps[:])

        # ---- EPS * colsum(V) (per output feature, on partitions) ----
        # xsum[c] = sum_k x[k, c]  (free-axis reduce of x^T rows)
        xsum_sb = sb.tile([128, CT], F32)
        for ci in range(CT):
            nc.vector.tensor_reduce(
                xsum_sb[:, ci:ci + 1], xT_sb[:, ci, :], axis=mybir.AxisListType.X,
                op=OP.add,
            )
        xsum_bf = sb.tile([128, CT], BF16)
        nc.vector.tensor_copy(xsum_bf[:], xsum_sb[:])
        vcol_sb = sb.tile([128, CT], F32)   # EPS * colsum(V), o on partitions
        vc_ps = med_ps.tile([128, CT], F32, tag="medps")
        for oj in range(CT):
            for ci in range(CT):
                nc.tensor.matmul(
                    vc_ps[:, oj:oj + 1],
                    wv_sb[:, ci, oj * 128:(oj + 1) * 128],
                    xsum_bf[:, ci:ci + 1],
                    start=(ci == 0), stop=(ci == CT - 1),
                )
        nc.vector.tensor_scalar_mul(vcol_sb[:], vc_ps[:], EPS)

        # ---- attention ----
        u_sb = sb.tile([128, H, N], BF16)       # exp(S - max) per head
        r_sb = sb.tile([128, H], F32)           # row sums
        rinv_sb = sb.tile([128, H], F32)
        mx_sb = sb.tile([128, H], F32)          # -max
        D_sb = sb.tile([128, N], BF16)          # cross-head sum of u/r
        lnD_sb = sb.tile([128, N], BF16)
        dinv_sb = sb.tile([128, N], BF16)

        for qc in range(QC):
            qs = slice(qc * 128, (qc + 1) * 128)
            for hh in range(H):
                oj, ph = hh // 2, (hh % 2) * 64
                s_ps = big_ps.tile([128, N], F32, tag="bigps")
                for nh in range(N // 512):
                    nc.tensor.matmul(
                        s_ps[:, nh * 512:(nh + 1) * 512],
                        QT_sb[ph:ph + 64, oj, qs],
                        KT_sb[ph:ph + 64, oj, nh * 512:(nh + 1) * 512],
                        start=True, stop=True,
                    )
                nc.vector.tensor_reduce(
                    mx_sb[:, hh:hh + 1], s_ps[:], axis=mybir.AxisListType.X,
                    op=OP.max, negate=True,
                )
                nc.scalar.activation(
                    u_sb[:, hh, :], s_ps[:], AF.Exp,
                    bias=mx_sb[:, hh:hh + 1], scale=1.0,
                    accum_out=r_sb[:, hh:hh + 1],
                )
            nc.vector.reciprocal(rinv_sb[:], r_sb[:])
            nc.vector.tensor_scalar(
                D_sb[:], u_sb[:, 0, :], rinv_sb[:, 0:1], None, op0=OP.mult,
            )
            for hh in range(1, H):
                nc.vector.scalar_tensor_tensor(
                    D_sb[:], u_sb[:, hh, :], rinv_sb[:, hh:hh + 1], D_sb[:],
                    op0=OP.mult, op1=OP.add,
                )
            nc.scalar.activation(lnD_sb[:], D_sb[:], AF.Ln)
            nc.scalar.activation(dinv_sb[:], lnD_sb[:], AF.Exp, scale=-1.0)

            ot_ps = ot_ps_pool.tile([128, CT, 128], F32, tag="otps")
            for hh in range(H):
                oj, ph = hh // 2, (hh % 2) * 64
                p_sb = pbuf.tile([128, N], BF16, tag="p")
                nc.vector.scalar_tensor_tensor(
                    p_sb[:], u_sb[:, hh, :], rinv_sb[:, hh:hh + 1], dinv_sb[:],
                    op0=OP.mult, op1=OP.mult,
                )
                pt_sb = ptbuf.tile([128, KT, 128], BF16, tag="pt")
                nc.sync.dma_start(pt_sb[:], p_sb[:], transpose=True)
                for kt in range(KT):
                    nc.tensor.matmul(
                        ot_ps[ph:ph + 64, oj, :],
                        V_sb[:, kt, hh * 64:(hh + 1) * 64],
                        pt_sb[:, kt, :],
                        start=(kt == 0), stop=(kt == KT - 1),
                    )

            ot_sb = pbuf.tile([128, CT, 128], BF16, tag="ot")
            for oj in range(CT):
                nc.scalar.activation(
                    ot_sb[:, oj, :], ot_ps[:, oj, :], AF.Identity,
                    bias=vcol_sb[:, oj:oj + 1],
                )
            y_ps = med_ps.tile([128, D], F32, tag="medps")
            for oj in range(CT):
                nc.tensor.matmul(
                    y_ps[:], ot_sb[:, oj, :], wo_sb[:, oj, :],
                    start=(oj == 0), stop=(oj == CT - 1),
                )
            y_sb = pbuf.tile([128, D], F32, tag="y")
            nc.vector.tensor_copy(y_sb[:], y_ps[:])
            nc.sync.dma_start(y.ap()[qc], y_sb[:])

    nc.compile()
    return nc


def _get_nc():
    global _CACHED_NC
    if _CACHED_NC is None:
        _CACHED_NC = build()
    return _CACHED_NC


def _prep_core_inputs(x, Wq, Wk, Wv, Wo):
    bf = ml_dtypes.bfloat16
    wq_h = np.ascontiguousarray((SCALE * Wq).T).astype(bf).reshape(CT, 128, D)
    wk_h = np.ascontiguousarray(Wk.T).astype(bf).reshape(CT, 128, D)
    wv_h = np.ascontiguousarray(Wv.T).astype(bf).reshape(CT, 128, D)
    wo_h = np.ascontiguousarray(Wo.T).astype(bf).reshape(CT, 128, D)
    in_maps = []
    for c in range(8):
        b, qh = c // 2, c % 2
        xT_b = np.ascontiguousarray(x[b].T).astype(bf)          # (512, 2048)
        xqT_b = np.ascontiguousarray(xT_b[:, qh * NQ:(qh + 1) * NQ])
        in_maps.append({
            "xT": xT_b.reshape(CT, 128, N),
            "xqT": xqT_b.reshape(CT, 128, NQ),
            "wq": wq_h, "wk": wk_h, "wv": wv_h, "wo": wo_h,
        })
    return in_maps


def kernel(x, Wq, Wk, Wv, Wo, _trace=False):
    nc = _get_nc()
    in_maps = _prep_core_inputs(x, Wq, Wk, Wv, Wo)
    res = run_bass_kernel_spmd(nc, in_maps, core_ids=list(range(8)), trace=_trace)
    out = np.empty((4, 2048, D), np.float32)
    for c in range(8):
        b, qh = c // 2, c % 2
        out[b, qh * NQ:(qh + 1) * NQ] = res.results[c]["y"].reshape(NQ, D)
    if _trace:
        return out, res
    return out
